# revision 25
# baseline (speedup 1.0000x reference)
"""Causal multi-head attention kernel for Trainium2 (Bass/Tile), 8-core SPMD.

Problem: bs=32 (batch*heads), n=2048, hs=128, fp32, causal mask.
Sharding: bs axis split across 8 cores (4 heads per core), no communication.

Per-head algorithm (flash-style, no running max — scores are ~N(0,1) so exp
is safe in fp32):
  S^T[k, q] = (K^T tile).T @ Q^T          (PE, fp32r, contraction over h=128)
  mask diagonal 128x128 blocks with -1e30  (DVE add of a precomputed tile)
  P^T = exp(S^T / sqrt(dk))               (ACT, PSUM -> SBUF, bf16 out)
  [O | denom] accumulated over k-tiles:    (PE, bf16)
      out[q, 0:128+1] += (P^T tile).T @ [V | 1]
  O_norm = O * (1/denom)                  (DVE reciprocal + tensor_scalar)

Layouts: Q^T, K^T ([h=128, n]) are prepared host-side by numpy transpose;
V_ext = [V | ones] in bf16 host-side. Causality of the mask input is verified
host-side (falls back to exact numpy if the mask is not causal).
"""

import math
import os
from contextlib import ExitStack

import numpy as np

BS, N, HS = 32, 2048, 128
NCORES = 8
HEADS_PER_CORE = BS // NCORES
P = 128                      # partitions / head-dim / k-tile
QB = 512                     # q block width for the S^T pass
NKT = N // P                 # 16 k-tiles per head
NQB = N // QB                # 4 q blocks per head
NQT = N // P                 # 16 q tiles per head
MASK_NEG = -1.0e30

# diag tile d = j % 4: (computed q-start within block, width, tri offset in tile)
# d<3: compute cols [128d, 512); d==3: compute [256, 512) (width 256 keeps
# fp32r at full rate; cols [256,384) are fully masked and never read by AV).
_DIAG = {0: (0, 512, 0), 1: (128, 384, 0), 2: (256, 256, 0), 3: (256, 256, 128)}


def _sblocks():
    """S^T tiles grouped into <=2-tile PSUM super-tile chunks per j.

    Returns (chunks, off, col): chunks is a list of chunk descriptors
    {tiles: [(j, b, qstart, width, diag, local0)], act_lo, act_hi, pt_col}
    where local0 is the tile's 512-aligned slot start inside the super-tile
    and [act_lo, act_hi) is the contiguous range one ACT exp covers.
    off[(j, b)] is the P^T slab column of that tile."""
    off = {}
    col = 0
    chunks = []
    for j in range(NKT):
        tiles = []
        for b in range(j // 4, NQB):
            if b == j // 4:
                d = j % 4
                qs, w, _ = _DIAG[d]
                qs += QB * b
                diag = True
            else:
                qs, w, diag = QB * b, QB, False
            tiles.append((j, b, qs, w, diag))
        for c0 in range(0, len(tiles), 2):
            group = tiles[c0 : c0 + 2]
            gtiles = []
            local = 0
            act_lo = None
            pt_col = col
            for (tj, tb, qs, w, diag) in group:
                local0 = local + (QB - w)   # right-aligned in its 512 slot
                if act_lo is None:
                    act_lo = local0
                gtiles.append((tj, tb, qs, w, diag, local0))
                off[(tj, tb)] = col
                col += w
                local += QB
            chunks.append(
                dict(tiles=gtiles, act_lo=act_lo, act_hi=local, pt_col=pt_col)
            )
    return chunks, off, col


def build_bass():
    import concourse.mybir as mybir
    import concourse.tile as tile
    from concourse import bacc

    nc = bacc.Bacc("TRN2", target_bir_lowering=False, debug=False, num_devices=8)
    f32 = mybir.dt.float32
    f32r = mybir.dt.float32r
    bf16 = mybir.dt.bfloat16

    qt_d = nc.dram_tensor("qt", [HEADS_PER_CORE, P, N], f32r, kind="ExternalInput")
    kt_d = nc.dram_tensor("kt", [HEADS_PER_CORE, P, N], f32r, kind="ExternalInput")
    v_d = nc.dram_tensor("vext", [HEADS_PER_CORE, N, HS + 1], bf16, kind="ExternalInput")
    out_d = nc.dram_tensor("out", [HEADS_PER_CORE, N, HS], f32, kind="ExternalOutput")

    scale = 1.0 / math.sqrt(float(HS))
    chunks, pt_off, pt_cols = _sblocks()

    with ExitStack() as ctx:
        tc = ctx.enter_context(tile.TileContext(nc))
        qt_pool = ctx.enter_context(tc.tile_pool(name="qt", bufs=2))
        kt_pool = ctx.enter_context(tc.tile_pool(name="kt", bufs=2))
        v_pool = ctx.enter_context(tc.tile_pool(name="vext", bufs=2))
        pt_pool = ctx.enter_context(tc.tile_pool(name="pt", bufs=2))
        o_pool = ctx.enter_context(tc.tile_pool(name="o", bufs=4))
        r_pool = ctx.enter_context(tc.tile_pool(name="recip", bufs=4))
        s_psum = ctx.enter_context(tc.tile_pool(name="spsum", bufs=3, space="PSUM"))
        o_psum = ctx.enter_context(tc.tile_pool(name="opsum", bufs=2, space="PSUM"))
        # s super-tiles are [128, 1024] = 2 banks x 3 bufs; o tiles 1 bank x 2

        def emit_loads(h):
            # chunked loads so compute can start before the full head arrives.
            # For head 0 the S pass (j=0) needs kt chunk 0 plus qt chunks in
            # order, so front-load the qt chunks.
            kt_c = [
                kt_pool.tile([P, QB], f32r, tag=f"kt{c}", name=f"kt{c}_{h}")
                for c in range(NQB)
            ]
            qt_c = [
                qt_pool.tile([P, QB], f32r, tag=f"qt{c}", name=f"qt{c}_{h}")
                for c in range(NQB)
            ]
            if h == 0:
                order = [("k", 0), ("q", 0), ("q", 1), ("q", 2), ("q", 3),
                         ("k", 1), ("k", 2), ("k", 3)]
            else:
                order = [(t, c) for c in range(NQB) for t in ("k", "q")]
            for (t, c) in order:
                dst = kt_c[c] if t == "k" else qt_c[c]
                src = kt_d if t == "k" else qt_d
                nc.sync.dma_start(dst[:], src.ap()[h, :, c * QB : (c + 1) * QB])
            v_c = []
            for c in range(2):
                v = v_pool.tile([P, NKT // 2, HS + 1], bf16, tag=f"v{c}")
                nc.sync.dma_start(
                    v[:],
                    v_d.ap()[h, c * (N // 2) : (c + 1) * (N // 2)].rearrange(
                        "(j p) c -> p j c", p=P
                    ),
                )
                v_c.append(v)
            return qt_c, kt_c, v_c

        def emit_s_chunk(ch, pt_t, qt_c, kt_c):
            for _one in [0]:
                s_t = s_psum.tile([P, 2 * QB], mybir.dt.float32)
                diag_zero = None
                for (j, b, qs, w, diag, l0) in ch["tiles"]:
                    nc.tensor.matmul(
                        s_t[:, l0 : l0 + w],
                        kt_c[j // 4][:, (j % 4) * P : (j % 4 + 1) * P],
                        qt_c[b][:, qs - b * QB : qs - b * QB + w],
                        start=True,
                        stop=True,
                    )
                    if diag:
                        diag_zero = pt_off[(j, b)] + _DIAG[j % 4][2]
                lo, hi = ch["act_lo"], ch["act_hi"]
                nc.scalar.activation(
                    pt_t[:, ch["pt_col"] : ch["pt_col"] + (hi - lo)],
                    s_t[:, lo:hi],
                    mybir.ActivationFunctionType.Exp,
                    scale=scale,
                )
                if diag_zero is not None:
                    # zero the strictly-upper triangle (k > q) of the exp'd
                    # diagonal block in SBUF on the otherwise-idle GpSimd
                    blk = pt_t[:, diag_zero : diag_zero + P]
                    nc.gpsimd.affine_select(
                        out=blk,
                        in_=blk,
                        compare_op=mybir.AluOpType.is_ge,
                        fill=0.0,
                        base=0,
                        pattern=[[1, P]],
                        channel_multiplier=-1,
                    )

        def emit_av_tile(h, t, pt_t, v_c):
            """AV + denom + normalize + store for one q-tile."""
            if True:
                b = t // 4
                o_t = o_psum.tile([P, HS + 1], mybir.dt.float32)
                for j in range(t + 1):
                    if b == j // 4:
                        qs = QB * b + _DIAG[j % 4][0]
                    else:
                        qs = QB * b
                    col = pt_off[(j, b)] + (P * t - qs)
                    nc.tensor.matmul(
                        o_t[:],
                        pt_t[:, col : col + P],
                        v_c[j // (NKT // 2)][:, j % (NKT // 2), :],
                        start=(j == 0),
                        stop=(j == t),
                    )
                recip = r_pool.tile([P, 1], mybir.dt.float32)
                nc.vector.reciprocal(recip[:], o_t[:, HS : HS + 1])
                o_sb = o_pool.tile([P, HS], mybir.dt.float32)
                nc.vector.tensor_scalar_mul(o_sb[:], o_t[:, :HS], recip[:])
                nc.sync.dma_start(out_d.ap()[h, t * P : (t + 1) * P], o_sb[:])

        # Intra-head interleave: emit AV q-tile t = j-1 between the S chunk
        # groups for j, so the in-order PE fills ACT-induced S stalls with AV
        # matmuls and each head's tail is just one AV tile.
        chunk_by_j = {}
        for ch in chunks:
            chunk_by_j.setdefault(ch["tiles"][0][0], []).append(ch)
        for h in range(HEADS_PER_CORE):
            qt_c, kt_c, v_c = emit_loads(h)
            pt_t = pt_pool.tile([P, pt_cols], bf16, tag="pt", name=f"pt_{h}")
            for j in range(NKT):
                for ch in chunk_by_j[j]:
                    emit_s_chunk(ch, pt_t, qt_c, kt_c)
                if j > 0:
                    emit_av_tile(h, j - 1, pt_t, v_c)
            emit_av_tile(h, NKT - 1, pt_t, v_c)

    nc.compile()
    return nc


_NC_CACHE = None


def _get_nc():
    global _NC_CACHE
    if _NC_CACHE is None:
        _NC_CACHE = build_bass()
    return _NC_CACHE


def _is_causal_mask(mask: np.ndarray) -> bool:
    if mask.shape != (BS, N, N) or mask.dtype != np.bool_:
        return False
    tri = np.triu(np.ones((N, N), dtype=np.bool_), k=1)
    if not np.array_equal(mask[0], tri):
        return False
    # all batch entries identical
    return bool((mask == mask[0]).all())


def _numpy_fallback(QW, KW, VW, dk, mask):
    out = np.empty((BS, N, HS), dtype=np.float32)
    inv = 1.0 / np.sqrt(np.float32(dk))
    for i in range(BS):
        s = (QW[i] @ KW[i].T) * inv
        s = np.where(mask[i], -np.inf, s)
        s = s - s.max(axis=-1, keepdims=True)
        e = np.exp(s)
        out[i] = (e @ VW[i]) / e.sum(axis=-1, keepdims=True)
    return out


def _prepare_in_maps(QW, KW, VW):
    import ml_dtypes

    in_maps = []
    for c in range(NCORES):
        sl = slice(c * HEADS_PER_CORE, (c + 1) * HEADS_PER_CORE)
        q = QW[sl]
        k = KW[sl]
        v = VW[sl]
        qt = np.ascontiguousarray(q.transpose(0, 2, 1))
        kt = np.ascontiguousarray(k.transpose(0, 2, 1))
        vext = np.empty((HEADS_PER_CORE, N, HS + 1), dtype=ml_dtypes.bfloat16)
        vext[:, :, :HS] = v.astype(ml_dtypes.bfloat16)
        vext[:, :, HS] = 1.0
        in_maps.append({"qt": qt, "kt": kt, "vext": vext})
    return in_maps


def _run(QW, KW, VW, trace=False, **spmd_kwargs):
    from concourse import bass_utils

    nc = _get_nc()
    in_maps = _prepare_in_maps(QW, KW, VW)
    res = bass_utils.run_bass_kernel_spmd(
        nc, in_maps, core_ids=list(range(NCORES)), trace=trace, **spmd_kwargs
    )
    out = np.concatenate([r["out"] for r in res.results], axis=0)
    return out, res


def kernel(QW, KW, VW, dk, mask):
    QW = np.asarray(QW, dtype=np.float32)
    KW = np.asarray(KW, dtype=np.float32)
    VW = np.asarray(VW, dtype=np.float32)
    mask = np.asarray(mask)
    if int(dk) != HS or not _is_causal_mask(mask):
        return _numpy_fallback(QW, KW, VW, int(dk), mask)
    out, _ = _run(QW, KW, VW, trace=bool(int(os.environ.get("KERNEL_TRACE", "0"))))
    return out


# revision 31
# speedup vs baseline: 1.0485x; 1.0485x over previous
"""Causal multi-head attention kernel for Trainium2 (Bass/Tile), 8-core SPMD.

Problem: bs=32 (batch*heads), n=2048, hs=128, fp32, causal mask.
Sharding: bs axis split across 8 cores (4 heads per core), no communication.

Per-head algorithm (flash-style, no running max — scores are ~N(0,1) so exp
is safe in fp32):
  S^T[k, q] = (K^T tile).T @ Q^T          (PE, fp32r, contraction over h=128)
  mask diagonal 128x128 blocks with -1e30  (DVE add of a precomputed tile)
  P^T = exp(S^T / sqrt(dk))               (ACT, PSUM -> SBUF, bf16 out)
  [O | denom] accumulated over k-tiles:    (PE, bf16)
      out[q, 0:128+1] += (P^T tile).T @ [V | 1]
  O_norm = O * (1/denom)                  (DVE reciprocal + tensor_scalar)

Layouts: Q^T, K^T ([h=128, n]) are prepared host-side by numpy transpose;
V_ext = [V | ones] in bf16 host-side. Causality of the mask input is verified
host-side (falls back to exact numpy if the mask is not causal).
"""

import math
import os
from contextlib import ExitStack

import numpy as np

BS, N, HS = 32, 2048, 128
NCORES = 8
HEADS_PER_CORE = BS // NCORES
P = 128                      # partitions / head-dim / k-tile
QB = 512                     # q block width for the S^T pass
NKT = N // P                 # 16 k-tiles per head
NQB = N // QB                # 4 q blocks per head
NQT = N // P                 # 16 q tiles per head
MASK_NEG = -1.0e30

# diag tile d = j % 4: (computed q-start within block, width, tri offset in tile)
# d<3: compute cols [128d, 512); d==3: compute [256, 512) (width 256 keeps
# fp32r at full rate; cols [256,384) are fully masked and never read by AV).
_DIAG = {0: (0, 512, 0), 1: (128, 384, 0), 2: (256, 256, 0), 3: (256, 256, 128)}


def _sblocks():
    """S^T tiles grouped into <=2-tile PSUM super-tile chunks per j.

    Returns (chunks, off, col): chunks is a list of chunk descriptors
    {tiles: [(j, b, qstart, width, diag, local0)], act_lo, act_hi, pt_col}
    where local0 is the tile's 512-aligned slot start inside the super-tile
    and [act_lo, act_hi) is the contiguous range one ACT exp covers.
    off[(j, b)] is the P^T slab column of that tile."""
    off = {}
    col = 0
    chunks = []
    for j in range(NKT):
        tiles = []
        for b in range(j // 4, NQB):
            if b == j // 4:
                d = j % 4
                qs, w, _ = _DIAG[d]
                qs += QB * b
                diag = True
            else:
                qs, w, diag = QB * b, QB, False
            tiles.append((j, b, qs, w, diag))
        for c0 in range(0, len(tiles), 2):
            group = tiles[c0 : c0 + 2]
            gtiles = []
            local = 0
            act_lo = None
            pt_col = col
            for (tj, tb, qs, w, diag) in group:
                local0 = local + (QB - w)   # right-aligned in its 512 slot
                if act_lo is None:
                    act_lo = local0
                gtiles.append((tj, tb, qs, w, diag, local0))
                off[(tj, tb)] = col
                col += w
                local += QB
            chunks.append(
                dict(tiles=gtiles, act_lo=act_lo, act_hi=local, pt_col=pt_col)
            )
    return chunks, off, col


def build_bass():
    import concourse.mybir as mybir
    import concourse.tile as tile
    from concourse import bacc

    nc = bacc.Bacc("TRN2", target_bir_lowering=False, debug=False, num_devices=8)
    f32 = mybir.dt.float32
    f32r = mybir.dt.float32r
    bf16 = mybir.dt.bfloat16

    qt_d = nc.dram_tensor("qt", [HEADS_PER_CORE, P, N], f32r, kind="ExternalInput")
    kt_d = nc.dram_tensor("kt", [HEADS_PER_CORE, P, N], f32r, kind="ExternalInput")
    v_d = nc.dram_tensor("vext", [HEADS_PER_CORE, N, HS + 1], bf16, kind="ExternalInput")
    out_d = nc.dram_tensor("out", [HEADS_PER_CORE, N, HS], f32, kind="ExternalOutput")

    scale = 1.0 / math.sqrt(float(HS))
    chunks, pt_off, pt_cols = _sblocks()

    with ExitStack() as ctx:
        tc = ctx.enter_context(tile.TileContext(nc))
        qt_pool = ctx.enter_context(tc.tile_pool(name="qt", bufs=2))
        kt_pool = ctx.enter_context(tc.tile_pool(name="kt", bufs=2))
        v_pool = ctx.enter_context(tc.tile_pool(name="vext", bufs=2))
        pt_pool = ctx.enter_context(tc.tile_pool(name="pt", bufs=2))
        o_pool = ctx.enter_context(tc.tile_pool(name="o", bufs=4))
        r_pool = ctx.enter_context(tc.tile_pool(name="recip", bufs=4))
        s_psum = ctx.enter_context(tc.tile_pool(name="spsum", bufs=3, space="PSUM"))
        o_psum = ctx.enter_context(tc.tile_pool(name="opsum", bufs=2, space="PSUM"))
        # s super-tiles are [128, 1024] = 2 banks x 3 bufs; o tiles 1 bank x 2

        def emit_loads(h):
            # chunked loads so compute can start before the full head arrives.
            # For head 0 the S pass (j=0) needs kt chunk 0 plus qt chunks in
            # order, so front-load the qt chunks.
            kt_c = [
                kt_pool.tile([P, QB], f32r, tag=f"kt{c}", name=f"kt{c}_{h}")
                for c in range(NQB)
            ]
            qt_c = [
                qt_pool.tile([P, QB], f32r, tag=f"qt{c}", name=f"qt{c}_{h}")
                for c in range(NQB)
            ]
            if h == 0:
                order = [("k", 0), ("q", 0), ("q", 1), ("q", 2), ("q", 3),
                         ("k", 1), ("k", 2), ("k", 3)]
            else:
                order = [(t, c) for c in range(NQB) for t in ("k", "q")]
            for (t, c) in order:
                dst = kt_c[c] if t == "k" else qt_c[c]
                src = kt_d if t == "k" else qt_d
                nc.sync.dma_start(dst[:], src.ap()[h, :, c * QB : (c + 1) * QB])
            v_c = []
            for c in range(2):
                v = v_pool.tile([P, NKT // 2, HS + 1], bf16, tag=f"v{c}")
                nc.sync.dma_start(
                    v[:],
                    v_d.ap()[h, c * (N // 2) : (c + 1) * (N // 2)].rearrange(
                        "(j p) c -> p j c", p=P
                    ),
                )
                v_c.append(v)
            return qt_c, kt_c, v_c

        def emit_s_chunk(ch, pt_t, qt_c, kt_c):
            for _one in [0]:
                s_t = s_psum.tile([P, 2 * QB], mybir.dt.float32)
                diag_zero = None
                for (j, b, qs, w, diag, l0) in ch["tiles"]:
                    nc.tensor.matmul(
                        s_t[:, l0 : l0 + w],
                        kt_c[j // 4][:, (j % 4) * P : (j % 4 + 1) * P],
                        qt_c[b][:, qs - b * QB : qs - b * QB + w],
                        start=True,
                        stop=True,
                    )
                    if diag:
                        diag_zero = pt_off[(j, b)] + _DIAG[j % 4][2]
                lo, hi = ch["act_lo"], ch["act_hi"]
                nc.scalar.activation(
                    pt_t[:, ch["pt_col"] : ch["pt_col"] + (hi - lo)],
                    s_t[:, lo:hi],
                    mybir.ActivationFunctionType.Exp,
                    scale=scale,
                )
                if diag_zero is not None:
                    # zero the strictly-upper triangle (k > q) of the exp'd
                    # diagonal block in SBUF on the otherwise-idle GpSimd
                    blk = pt_t[:, diag_zero : diag_zero + P]
                    nc.gpsimd.affine_select(
                        out=blk,
                        in_=blk,
                        compare_op=mybir.AluOpType.is_ge,
                        fill=0.0,
                        base=0,
                        pattern=[[1, P]],
                        channel_multiplier=-1,
                    )

        def emit_av_tile(h, t, pt_t, v_c):
            """AV + denom + normalize + store for one q-tile."""
            if True:
                b = t // 4
                o_t = o_psum.tile([P, HS + 1], mybir.dt.float32)
                for j in range(t + 1):
                    if b == j // 4:
                        qs = QB * b + _DIAG[j % 4][0]
                    else:
                        qs = QB * b
                    col = pt_off[(j, b)] + (P * t - qs)
                    nc.tensor.matmul(
                        o_t[:],
                        pt_t[:, col : col + P],
                        v_c[j // (NKT // 2)][:, j % (NKT // 2), :],
                        start=(j == 0),
                        stop=(j == t),
                    )
                recip = r_pool.tile([P, 1], mybir.dt.float32)
                nc.vector.reciprocal(recip[:], o_t[:, HS : HS + 1])
                o_sb = o_pool.tile([P, HS], mybir.dt.float32)
                nc.vector.tensor_scalar_mul(o_sb[:], o_t[:, :HS], recip[:])
                nc.sync.dma_start(out_d.ap()[h, t * P : (t + 1) * P], o_sb[:])

        # Cross-head interleave: head h-1's AV q-tiles are spread between head
        # h's S chunks, so the in-order PE fills ACT-induced S stalls with AV
        # matmuls whose exp inputs are long since available.
        nchunks = len(chunks)
        av_prev = None
        for h in range(HEADS_PER_CORE):
            qt_c, kt_c, v_c = emit_loads(h)
            pt_t = pt_pool.tile([P, pt_cols], bf16, tag="pt", name=f"pt_{h}")
            done_av = 0
            for i, ch in enumerate(chunks):
                emit_s_chunk(ch, pt_t, qt_c, kt_c)
                if av_prev is not None:
                    ph, ppt, pv = av_prev
                    while done_av < NQT and done_av * nchunks < (i + 1) * NQT:
                        emit_av_tile(ph, done_av, ppt, pv)
                        done_av += 1
            if av_prev is not None:
                ph, ppt, pv = av_prev
                while done_av < NQT:
                    emit_av_tile(ph, done_av, ppt, pv)
                    done_av += 1
            av_prev = (h, pt_t, v_c)
        ph, ppt, pv = av_prev
        for t in range(NQT):
            emit_av_tile(ph, t, ppt, pv)

    nc.compile()
    return nc


_NC_CACHE = None


def _get_nc():
    global _NC_CACHE
    if _NC_CACHE is None:
        _NC_CACHE = build_bass()
    return _NC_CACHE


def _is_causal_mask(mask: np.ndarray) -> bool:
    if mask.shape != (BS, N, N) or mask.dtype != np.bool_:
        return False
    tri = np.triu(np.ones((N, N), dtype=np.bool_), k=1)
    if not np.array_equal(mask[0], tri):
        return False
    # all batch entries identical
    return bool((mask == mask[0]).all())


def _numpy_fallback(QW, KW, VW, dk, mask):
    out = np.empty((BS, N, HS), dtype=np.float32)
    inv = 1.0 / np.sqrt(np.float32(dk))
    for i in range(BS):
        s = (QW[i] @ KW[i].T) * inv
        s = np.where(mask[i], -np.inf, s)
        s = s - s.max(axis=-1, keepdims=True)
        e = np.exp(s)
        out[i] = (e @ VW[i]) / e.sum(axis=-1, keepdims=True)
    return out


def _prepare_in_maps(QW, KW, VW):
    import ml_dtypes

    in_maps = []
    for c in range(NCORES):
        sl = slice(c * HEADS_PER_CORE, (c + 1) * HEADS_PER_CORE)
        q = QW[sl]
        k = KW[sl]
        v = VW[sl]
        qt = np.ascontiguousarray(q.transpose(0, 2, 1))
        kt = np.ascontiguousarray(k.transpose(0, 2, 1))
        vext = np.empty((HEADS_PER_CORE, N, HS + 1), dtype=ml_dtypes.bfloat16)
        vext[:, :, :HS] = v.astype(ml_dtypes.bfloat16)
        vext[:, :, HS] = 1.0
        in_maps.append({"qt": qt, "kt": kt, "vext": vext})
    return in_maps


def _run(QW, KW, VW, trace=False, **spmd_kwargs):
    from concourse import bass_utils

    nc = _get_nc()
    in_maps = _prepare_in_maps(QW, KW, VW)
    res = bass_utils.run_bass_kernel_spmd(
        nc, in_maps, core_ids=list(range(NCORES)), trace=trace, **spmd_kwargs
    )
    out = np.concatenate([r["out"] for r in res.results], axis=0)
    return out, res


def kernel(QW, KW, VW, dk, mask):
    QW = np.asarray(QW, dtype=np.float32)
    KW = np.asarray(KW, dtype=np.float32)
    VW = np.asarray(VW, dtype=np.float32)
    mask = np.asarray(mask)
    if int(dk) != HS or not _is_causal_mask(mask):
        return _numpy_fallback(QW, KW, VW, int(dk), mask)
    out, _ = _run(QW, KW, VW, trace=bool(int(os.environ.get("KERNEL_TRACE", "0"))))
    return out
